# revision 30
# baseline (speedup 1.0000x reference)
"""Trainium2 Bass kernel for nn_CubicalModel_ISM.

Reference computation:
    Xp = reshape(I1 @ p0, (28, 28)); Yp = reshape(I2 @ p1, (28, 28))
    dgm1 = Xp[inds1[0::2], inds1[1::2]].reshape(50, 2)
    dgm2 = Yp[inds2[0::2], inds2[1::2]].reshape(50, 2)

Only the <=100 gathered rows of each 784-row GEMV are live, and the gather
commutes with the per-row dot product.  So the host selects the 100 indexed
rows of I1 and of I2 (the "tiny gather", applied to the input instead of the
output), the device computes the 200 surviving dot products of length 32768
with k sharded over the 8 cores (3.3 MB of HBM traffic per core), and the
host sums the 8 partial vectors (the k-unshard) and reshapes.

Precision/speed: plain fp32 matmuls stream at 1/4 PE rate and fp32r loses
~1e-4; instead every fp32 operand is split hi+lo into two fp16 halves
(22 mantissa bits total) and the product expanded as
    A.q ~= Ahi.qhi + Ahi.qlo + Alo.qhi        (the lo.lo term is ~2^-22)
with all three terms as full-rate fp16 matmuls accumulating into fp32 PSUM.
Same total HBM bytes as fp32 (2 x 2-byte halves).

Per-core device program: the core's 4096 k-rows are split into 8 tiles of
four 128-row k-chunks (one 413 KB DMA per tile, 3232 B lines, issue
alternating between the two HWDGE engines SP/ACT so descriptor submission
is parallel and all issues fit the queue rings early).  Tile columns, per
SBUF partition p (k within chunk), chunks c0..c3 = 4t..4t+3:

    cols    0:400   hi halves, chunks c0,c1   (per chunk: I1 100 | I2 100)
    cols  400:800   hi halves, chunks c2,c3
    cols  800:1200  lo halves, chunks c0,c1
    cols 1200:1600  lo halves, chunks c2,c3
    cols 1600:1604  q hi (p0[c0], p1[c0], p0[c1], p1[c1])
    cols 1604:1608  q lo (same order)
    cols 1608:1616  same two q groups for chunks c2,c3

Four matmuls per tile accumulate into two persistent PSUM [8, 400] tiles
(one per chunk-pair parity):
    mA: lhsT = q hi+lo (8 cols), rhs = hi matrix (400 cols)
        rows 0-3 += qhi.Ahi       rows 4-7 += qlo.Ahi
    mB: lhsT = q hi (4 cols),    rhs = lo matrix (400 cols)
        rows 0-3 += qhi.Alo
Useful segments of each PSUM tile (first/second chunk of each pair):
    dgm1 partials: rows {0,4}[0:100]   and rows {2,6}[200:300]
    dgm2 partials: rows {1,5}[100:200] and rows {3,7}[300:400]
Off-segment entries accumulate garbage cross-terms; never read.  The host
adds the segments and reduces across the 8 cores.
"""

import numpy as np

K = 32768
NCORES = 8
KS = K // NCORES          # 4096 k columns per core
T = KS // 512             # 8 tiles of 4 k-chunks
TW = 1616                 # 16 blocks of 100 fp16 matrix cols + 16 q cols
R = 100                   # gathered rows per diagram
SIDE = 28

_cache = {}


def _build_nc():
    import concourse.bacc as bacc
    import concourse.mybir as mybir
    from concourse.tile import TileContext

    f32 = mybir.dt.float32
    f16 = mybir.dt.float16
    nc = bacc.Bacc("TRN2", target_bir_lowering=False, debug=False,
                   num_devices=NCORES)
    # two logical tiles per DMA: 826 KB transfers with 6464 B lines keep
    # both HWDGE queue rings saturated from the first issue
    a = nc.declare_dram_parameter("a", [T // 2, 128, 2 * TW], f16,
                                  isOutput=False)
    y = nc.declare_dram_parameter("y", [8, 800], f32, isOutput=True)

    with TileContext(nc) as tc:
        with (
            tc.tile_pool(name="apool", bufs=T // 2) as apool,
            tc.tile_pool(name="opool", bufs=1) as opool,
            tc.tile_pool(name="ps", bufs=2, space="PSUM") as pspool,
        ):
            # full-partition tiles so each lands at PSUM base partition 0
            # (matmul output base partition must be 0/32/64/96)
            ps_a = pspool.tile([128, 400], f32, name="ps_a", tag="ps_a")[0:8, :]
            ps_b = pspool.tile([128, 400], f32, name="ps_b", tag="ps_b")[0:8, :]

            # PE_HAM releases the clock gate (1.2 -> 2.4 GHz) only after
            # ~3.4 us of sustained PE activity.  Spin dummy matmuls on a
            # zeroed tile while the first DMAs are in flight so the real
            # matmuls run warm.
            warm = apool.tile([128, 512], f16, name="warm", tag="warm")
            nc.gpsimd.memset(warm, 0.0)
            ps_w = pspool.tile([128, 512], f32, name="ps_w", tag="ps_w")[0:8, :]
            for _ in range(10):
                nc.tensor.matmul(ps_w, warm[:, 0:8], warm[:, 0:512],
                                 start=True, stop=True)
            for t in range(T // 2):
                at = apool.tile([128, 2 * TW], f16)
                eng = nc.sync if t % 2 == 0 else nc.scalar
                eng.dma_start(out=at, in_=a[t])

                for s in range(2):
                    base = s * TW

                    def mA(ps, qoff, rhs0, start=False, stop=False):
                        nc.tensor.matmul(
                            ps, at[:, base + qoff:base + qoff + 8],
                            at[:, base + rhs0:base + rhs0 + 400],
                            start=start, stop=stop)

                    def mB(ps, qoff, rhs0):
                        nc.tensor.matmul(
                            ps[0:4, :], at[:, base + qoff:base + qoff + 4],
                            at[:, base + rhs0:base + rhs0 + 400],
                            start=False, stop=False)

                    # ps_a <- even chunk pair of the sub-tile; ps_b <- odd.
                    # Per PSUM bank the first matmul carries start=True and
                    # the last carries stop=True (and must be emitted last).
                    first = t == 0 and s == 0
                    lastt = t == T // 2 - 1 and s == 1
                    if lastt:
                        mB(ps_a, 1600, 800)
                        mA(ps_a, 1600, 0, stop=True)
                        mB(ps_b, 1608, 1200)
                        mA(ps_b, 1608, 400, stop=True)
                    else:
                        mA(ps_a, 1600, 0, start=first)
                        mB(ps_a, 1600, 800)
                        mA(ps_b, 1608, 400, start=first)
                        mB(ps_b, 1608, 1200)

            yt = opool.tile([8, 800], f32)
            nc.vector.tensor_copy(out=yt[:, 0:400], in_=ps_a)
            nc.vector.tensor_copy(out=yt[:, 400:800], in_=ps_b)
            nc.sync.dma_start(out=y[:], in_=yt)
    nc.compile()
    return nc


def _split16(x):
    hi = x.astype(np.float16)
    lo = (x - hi.astype(np.float32)).astype(np.float16)
    return hi, lo


def _prep_inputs(p0, p1, I1, I2, inds1, inds2):
    idx1 = inds1.astype(np.int64).reshape(-1, 2)
    idx2 = inds2.astype(np.int64).reshape(-1, 2)
    rows1 = idx1[:, 0] * SIDE + idx1[:, 1]      # flat positions, in order
    rows2 = idx2[:, 0] * SIDE + idx2[:, 1]

    selT = np.empty((K, 2 * R), np.float32)
    selT[:, 0:R] = I1[rows1, :].T
    selT[:, R:2 * R] = I2[rows2, :].T
    sel_hi, sel_lo = _split16(selT)             # [K, 200] each
    q = np.stack([p0, p1], axis=-1)             # [K, 2]
    q_hi, q_lo = _split16(q)

    in_maps = []
    for cix in range(NCORES):
        o = cix * KS
        bh = sel_hi[o:o + KS].reshape(T, 4, 128, 2 * R)
        bl = sel_lo[o:o + KS].reshape(T, 4, 128, 2 * R)
        qh = q_hi[o:o + KS].reshape(T, 2, 2, 128, 2)
        ql = q_lo[o:o + KS].reshape(T, 2, 2, 128, 2)
        a = np.empty((T, 128, TW), np.float16)
        a[:, :, 0:200] = bh[:, 0]
        a[:, :, 200:400] = bh[:, 1]
        a[:, :, 400:600] = bh[:, 2]
        a[:, :, 600:800] = bh[:, 3]
        a[:, :, 800:1000] = bl[:, 0]
        a[:, :, 1000:1200] = bl[:, 1]
        a[:, :, 1200:1400] = bl[:, 2]
        a[:, :, 1400:1600] = bl[:, 3]
        # q groups: [1600:1604] = qhi pair0, [1604:1608] = qlo pair0,
        #           [1608:1612] = qhi pair1, [1612:1616] = qlo pair1
        a[:, :, 1600:1602] = qh[:, 0, 0]
        a[:, :, 1602:1604] = qh[:, 0, 1]
        a[:, :, 1604:1606] = ql[:, 0, 0]
        a[:, :, 1606:1608] = ql[:, 0, 1]
        a[:, :, 1608:1610] = qh[:, 1, 0]
        a[:, :, 1610:1612] = qh[:, 1, 1]
        a[:, :, 1612:1614] = ql[:, 1, 0]
        a[:, :, 1614:1616] = ql[:, 1, 1]
        # pack two logical tiles per DMA tile: [T, 128, TW] ->
        # [T//2, 128, 2*TW] with tile 2u at cols 0:TW, tile 2u+1 at TW:2*TW
        a2 = np.ascontiguousarray(
            a.reshape(T // 2, 2, 128, TW).transpose(0, 2, 1, 3)
        ).reshape(T // 2, 128, 2 * TW)
        in_maps.append({"a": a2})
    return in_maps


def _run(in_maps, trace=False):
    from concourse.bass_utils import run_bass_kernel_spmd

    if "nc" not in _cache:
        _cache["nc"] = _build_nc()
    return run_bass_kernel_spmd(
        _cache["nc"], in_maps, list(range(NCORES)), trace=trace
    )


def kernel(p0, p1, I1, I2, inds1, inds2):
    p0 = np.ascontiguousarray(np.asarray(p0, dtype=np.float32))
    p1 = np.ascontiguousarray(np.asarray(p1, dtype=np.float32))
    I1 = np.asarray(I1, dtype=np.float32)
    I2 = np.asarray(I2, dtype=np.float32)
    inds1 = np.asarray(inds1)
    inds2 = np.asarray(inds2)

    in_maps = _prep_inputs(p0, p1, I1, I2, inds1, inds2)
    results = _run(in_maps).results

    acc = np.zeros((2, R), np.float64)
    for r in results:
        yf = r["y"].astype(np.float64)
        for half in range(2):
            yc = yf[:, half * 400:(half + 1) * 400]
            acc[0] += (yc[0, 0:100] + yc[4, 0:100]
                       + yc[2, 200:300] + yc[6, 200:300])      # dgm1
            acc[1] += (yc[1, 100:200] + yc[5, 100:200]
                       + yc[3, 300:400] + yc[7, 300:400])      # dgm2
    vals = acc.astype(np.float32)
    dgm1 = vals[0].reshape(R // 2, 2)
    dgm2 = vals[1].reshape(R // 2, 2)
    return (dgm1, dgm2)


# revision 32
# speedup vs baseline: 1.1781x; 1.1781x over previous
"""Trainium2 Bass kernel for nn_CubicalModel_ISM.

Reference computation:
    Xp = reshape(I1 @ p0, (28, 28)); Yp = reshape(I2 @ p1, (28, 28))
    dgm1 = Xp[inds1[0::2], inds1[1::2]].reshape(50, 2)
    dgm2 = Yp[inds2[0::2], inds2[1::2]].reshape(50, 2)

Only the <=100 gathered rows of each 784-row GEMV are live, and the gather
commutes with the per-row dot product.  So the host selects the 100 indexed
rows of I1 and of I2 (the "tiny gather", applied to the input instead of the
output), the device computes the 200 surviving dot products of length 32768
with k sharded over the 8 cores (3.3 MB of HBM traffic per core), and the
host sums the 8 partial vectors (the k-unshard) and reshapes.

Precision/speed: plain fp32 matmuls stream at 1/4 PE rate and fp32r loses
~1e-4; instead every fp32 operand is split hi+lo into two fp16 halves
(22 mantissa bits total) and the product expanded as
    A.q ~= Ahi.qhi + Ahi.qlo + Alo.qhi        (the lo.lo term is ~2^-22)
with all three terms as full-rate fp16 matmuls accumulating into fp32 PSUM.
Same total HBM bytes as fp32 (2 x 2-byte halves).

Per-core device program: the core's 4096 k-rows are split into 8 tiles of
four 128-row k-chunks (one 413 KB DMA per tile, 3232 B lines, issue
alternating between the two HWDGE engines SP/ACT so descriptor submission
is parallel and all issues fit the queue rings early).  Tile columns, per
SBUF partition p (k within chunk), chunks c0..c3 = 4t..4t+3:

    cols    0:400   hi halves, chunks c0,c1   (per chunk: I1 100 | I2 100)
    cols  400:800   hi halves, chunks c2,c3
    cols  800:1200  lo halves, chunks c0,c1
    cols 1200:1600  lo halves, chunks c2,c3
    cols 1600:1604  q hi (p0[c0], p1[c0], p0[c1], p1[c1])
    cols 1604:1608  q lo (same order)
    cols 1608:1616  same two q groups for chunks c2,c3

Four matmuls per tile accumulate into two persistent PSUM [8, 400] tiles
(one per chunk-pair parity):
    mA: lhsT = q hi+lo (8 cols), rhs = hi matrix (400 cols)
        rows 0-3 += qhi.Ahi       rows 4-7 += qlo.Ahi
    mB: lhsT = q hi (4 cols),    rhs = lo matrix (400 cols)
        rows 0-3 += qhi.Alo
Useful segments of each PSUM tile (first/second chunk of each pair):
    dgm1 partials: rows {0,4}[0:100]   and rows {2,6}[200:300]
    dgm2 partials: rows {1,5}[100:200] and rows {3,7}[300:400]
Off-segment entries accumulate garbage cross-terms; never read.  The host
adds the segments and reduces across the 8 cores.
"""

import numpy as np

K = 32768
NCORES = 8
KS = K // NCORES          # 4096 k columns per core
T = KS // 512             # 8 tiles of 4 k-chunks
TW = 1616                 # 16 blocks of 100 fp16 matrix cols + 16 q cols
R = 100                   # gathered rows per diagram
SIDE = 28

_cache = {}


def _build_nc():
    import concourse.bacc as bacc
    import concourse.mybir as mybir
    from concourse.tile import TileContext

    f32 = mybir.dt.float32
    f16 = mybir.dt.float16
    f8 = mybir.dt.float8e5
    nc = bacc.Bacc("TRN2", target_bir_lowering=False, debug=False,
                   num_devices=NCORES)
    # hi halves + q columns in fp16, lo correction halves in fp8e5m2
    # (the correction only needs a few bits of its 2^-12-scale precision)
    # -> 2.5 MB/core instead of 3.3.  Two logical tiles per DMA keep both
    # HWDGE queue rings saturated from the first issue.
    HW2 = 816                 # per logical tile: 800 hi cols + 16 q cols
    LW2 = 808                 # per logical tile: 800 lo cols + 8 q cols
    ah = nc.declare_dram_parameter("ah", [T // 2, 128, 2 * HW2], f16,
                                   isOutput=False)
    al = nc.declare_dram_parameter("al", [T // 2, 128, 2 * LW2], f8,
                                   isOutput=False)
    y = nc.declare_dram_parameter("y", [8, 800], f32, isOutput=True)

    with TileContext(nc) as tc:
        with (
            tc.tile_pool(name="hpool", bufs=T // 2) as hpool,
            tc.tile_pool(name="lpool", bufs=T // 2) as lpool,
            tc.tile_pool(name="opool", bufs=1) as opool,
            tc.tile_pool(name="ps", bufs=2, space="PSUM") as pspool,
        ):
            # full-partition tiles so each lands at PSUM base partition 0
            # (matmul output base partition must be 0/32/64/96)
            ps_a = pspool.tile([128, 400], f32, name="ps_a", tag="ps_a")[0:8, :]
            ps_b = pspool.tile([128, 400], f32, name="ps_b", tag="ps_b")[0:8, :]

            # PE_HAM releases the clock gate (1.2 -> 2.4 GHz) only after
            # ~3.4 us of sustained PE activity.  Spin dummy matmuls on a
            # zeroed tile while the first DMAs are in flight so the real
            # matmuls run warm.
            warm = opool.tile([128, 512], f16, name="warm", tag="warm")
            nc.gpsimd.memset(warm, 0.0)
            ps_w = pspool.tile([128, 512], f32, name="ps_w", tag="ps_w")[0:8, :]
            for _ in range(10):
                nc.tensor.matmul(ps_w, warm[:, 0:8], warm[:, 0:512],
                                 start=True, stop=True)
            for t in range(T // 2):
                ath = hpool.tile([128, 2 * HW2], f16)
                atl = lpool.tile([128, 2 * LW2], f8)
                enh = nc.sync if t % 2 == 0 else nc.scalar
                enl = nc.scalar if t % 2 == 0 else nc.sync
                enh.dma_start(out=ath, in_=ah[t])
                enl.dma_start(out=atl, in_=al[t])

                for s in range(2):
                    hb = s * HW2
                    lb = s * LW2

                    def mA(ps, pair, start=False, stop=False):
                        nc.tensor.matmul(
                            ps, ath[:, hb + 800 + 8 * pair:hb + 808 + 8 * pair],
                            ath[:, hb + 400 * pair:hb + 400 * pair + 400],
                            start=start, stop=stop)

                    def mB(ps, pair):
                        nc.tensor.matmul(
                            ps[0:4, :],
                            atl[:, lb + 800 + 4 * pair:lb + 804 + 4 * pair],
                            atl[:, lb + 400 * pair:lb + 400 * pair + 400],
                            start=False, stop=False)

                    # ps_a <- even chunk pair of the sub-tile; ps_b <- odd.
                    # Per PSUM bank the first matmul carries start=True and
                    # the last carries stop=True (and must be emitted last).
                    first = t == 0 and s == 0
                    lastt = t == T // 2 - 1 and s == 1
                    if lastt:
                        mB(ps_a, 0)
                        mA(ps_a, 0, stop=True)
                        mB(ps_b, 1)
                        mA(ps_b, 1, stop=True)
                    else:
                        mA(ps_a, 0, start=first)
                        mB(ps_a, 0)
                        mA(ps_b, 1, start=first)
                        mB(ps_b, 1)

            yt = opool.tile([8, 800], f32)
            nc.vector.tensor_copy(out=yt[:, 0:400], in_=ps_a)
            nc.vector.tensor_copy(out=yt[:, 400:800], in_=ps_b)
            nc.sync.dma_start(out=y[:], in_=yt)
    nc.compile()
    return nc


def _split16(x):
    hi = x.astype(np.float16)
    lo = (x - hi.astype(np.float32)).astype(np.float16)
    return hi, lo


def _prep_inputs(p0, p1, I1, I2, inds1, inds2):
    idx1 = inds1.astype(np.int64).reshape(-1, 2)
    idx2 = inds2.astype(np.int64).reshape(-1, 2)
    rows1 = idx1[:, 0] * SIDE + idx1[:, 1]      # flat positions, in order
    rows2 = idx2[:, 0] * SIDE + idx2[:, 1]

    import ml_dtypes

    HW2, LW2 = 816, 808
    selT = np.empty((K, 2 * R), np.float32)
    selT[:, 0:R] = I1[rows1, :].T
    selT[:, R:2 * R] = I2[rows2, :].T
    sel_hi = selT.astype(np.float16)
    sel_lo8 = (selT - sel_hi.astype(np.float32)).astype(ml_dtypes.float8_e5m2)
    q = np.stack([p0, p1], axis=-1)             # [K, 2]
    q_hi, q_lo = _split16(q)
    q_hi8 = q.astype(ml_dtypes.float8_e5m2)

    in_maps = []
    for cix in range(NCORES):
        o = cix * KS
        bh = sel_hi[o:o + KS].reshape(T, 4, 128, 2 * R)
        bl = sel_lo8[o:o + KS].reshape(T, 4, 128, 2 * R)
        qh = q_hi[o:o + KS].reshape(T, 2, 2, 128, 2)
        ql = q_lo[o:o + KS].reshape(T, 2, 2, 128, 2)
        q8 = q_hi8[o:o + KS].reshape(T, 2, 2, 128, 2)

        ah = np.empty((T, 128, HW2), np.float16)
        ah[:, :, 0:200] = bh[:, 0]
        ah[:, :, 200:400] = bh[:, 1]
        ah[:, :, 400:600] = bh[:, 2]
        ah[:, :, 600:800] = bh[:, 3]
        # q groups: [800:804] = qhi pair0, [804:808] = qlo pair0,
        #           [808:812] = qhi pair1, [812:816] = qlo pair1
        ah[:, :, 800:802] = qh[:, 0, 0]
        ah[:, :, 802:804] = qh[:, 0, 1]
        ah[:, :, 804:806] = ql[:, 0, 0]
        ah[:, :, 806:808] = ql[:, 0, 1]
        ah[:, :, 808:810] = qh[:, 1, 0]
        ah[:, :, 810:812] = qh[:, 1, 1]
        ah[:, :, 812:814] = ql[:, 1, 0]
        ah[:, :, 814:816] = ql[:, 1, 1]

        alr = np.empty((T, 128, LW2), ml_dtypes.float8_e5m2)
        alr[:, :, 0:200] = bl[:, 0]
        alr[:, :, 200:400] = bl[:, 1]
        alr[:, :, 400:600] = bl[:, 2]
        alr[:, :, 600:800] = bl[:, 3]
        alr[:, :, 800:802] = q8[:, 0, 0]
        alr[:, :, 802:804] = q8[:, 0, 1]
        alr[:, :, 804:806] = q8[:, 1, 0]
        alr[:, :, 806:808] = q8[:, 1, 1]

        # pack two logical tiles per DMA tile
        ah2 = np.ascontiguousarray(
            ah.reshape(T // 2, 2, 128, HW2).transpose(0, 2, 1, 3)
        ).reshape(T // 2, 128, 2 * HW2)
        al2 = np.ascontiguousarray(
            alr.reshape(T // 2, 2, 128, LW2).transpose(0, 2, 1, 3)
        ).reshape(T // 2, 128, 2 * LW2)
        in_maps.append({"ah": ah2, "al": al2})
    return in_maps


def _run(in_maps, trace=False):
    from concourse.bass_utils import run_bass_kernel_spmd

    if "nc" not in _cache:
        _cache["nc"] = _build_nc()
    return run_bass_kernel_spmd(
        _cache["nc"], in_maps, list(range(NCORES)), trace=trace
    )


def kernel(p0, p1, I1, I2, inds1, inds2):
    p0 = np.ascontiguousarray(np.asarray(p0, dtype=np.float32))
    p1 = np.ascontiguousarray(np.asarray(p1, dtype=np.float32))
    I1 = np.asarray(I1, dtype=np.float32)
    I2 = np.asarray(I2, dtype=np.float32)
    inds1 = np.asarray(inds1)
    inds2 = np.asarray(inds2)

    in_maps = _prep_inputs(p0, p1, I1, I2, inds1, inds2)
    results = _run(in_maps).results

    acc = np.zeros((2, R), np.float64)
    for r in results:
        yf = r["y"].astype(np.float64)
        for half in range(2):
            yc = yf[:, half * 400:(half + 1) * 400]
            acc[0] += (yc[0, 0:100] + yc[4, 0:100]
                       + yc[2, 200:300] + yc[6, 200:300])      # dgm1
            acc[1] += (yc[1, 100:200] + yc[5, 100:200]
                       + yc[3, 300:400] + yc[7, 300:400])      # dgm2
    vals = acc.astype(np.float32)
    dgm1 = vals[0].reshape(R // 2, 2)
    dgm2 = vals[1].reshape(R // 2, 2)
    return (dgm1, dgm2)


# revision 33
# speedup vs baseline: 1.1861x; 1.0068x over previous
"""Trainium2 Bass kernel for nn_CubicalModel_ISM.

Reference computation:
    Xp = reshape(I1 @ p0, (28, 28)); Yp = reshape(I2 @ p1, (28, 28))
    dgm1 = Xp[inds1[0::2], inds1[1::2]].reshape(50, 2)
    dgm2 = Yp[inds2[0::2], inds2[1::2]].reshape(50, 2)

Only the <=100 gathered rows of each 784-row GEMV are live, and the gather
commutes with the per-row dot product.  So the host selects the 100 indexed
rows of I1 and of I2 (the "tiny gather", applied to the input instead of the
output), the device computes the 200 surviving dot products of length 32768
with k sharded over the 8 cores (3.3 MB of HBM traffic per core), and the
host sums the 8 partial vectors (the k-unshard) and reshapes.

Precision/speed: plain fp32 matmuls stream at 1/4 PE rate and fp32r loses
~1e-4; instead every fp32 operand is split hi+lo into two fp16 halves
(22 mantissa bits total) and the product expanded as
    A.q ~= Ahi.qhi + Ahi.qlo + Alo.qhi        (the lo.lo term is ~2^-22)
with all three terms as full-rate fp16 matmuls accumulating into fp32 PSUM.
Same total HBM bytes as fp32 (2 x 2-byte halves).

Per-core device program: the core's 4096 k-rows are split into 8 tiles of
four 128-row k-chunks (one 413 KB DMA per tile, 3232 B lines, issue
alternating between the two HWDGE engines SP/ACT so descriptor submission
is parallel and all issues fit the queue rings early).  Tile columns, per
SBUF partition p (k within chunk), chunks c0..c3 = 4t..4t+3:

    cols    0:400   hi halves, chunks c0,c1   (per chunk: I1 100 | I2 100)
    cols  400:800   hi halves, chunks c2,c3
    cols  800:1200  lo halves, chunks c0,c1
    cols 1200:1600  lo halves, chunks c2,c3
    cols 1600:1604  q hi (p0[c0], p1[c0], p0[c1], p1[c1])
    cols 1604:1608  q lo (same order)
    cols 1608:1616  same two q groups for chunks c2,c3

Four matmuls per tile accumulate into two persistent PSUM [8, 400] tiles
(one per chunk-pair parity):
    mA: lhsT = q hi+lo (8 cols), rhs = hi matrix (400 cols)
        rows 0-3 += qhi.Ahi       rows 4-7 += qlo.Ahi
    mB: lhsT = q hi (4 cols),    rhs = lo matrix (400 cols)
        rows 0-3 += qhi.Alo
Useful segments of each PSUM tile (first/second chunk of each pair):
    dgm1 partials: rows {0,4}[0:100]   and rows {2,6}[200:300]
    dgm2 partials: rows {1,5}[100:200] and rows {3,7}[300:400]
Off-segment entries accumulate garbage cross-terms; never read.  The host
adds the segments and reduces across the 8 cores.
"""

import numpy as np

K = 32768
NCORES = 8
KS = K // NCORES          # 4096 k columns per core
T = KS // 512             # 8 tiles of 4 k-chunks
TW = 1616                 # 16 blocks of 100 fp16 matrix cols + 16 q cols
R = 100                   # gathered rows per diagram
SIDE = 28

_cache = {}


def _build_nc():
    import concourse.bacc as bacc
    import concourse.mybir as mybir
    from concourse.tile import TileContext

    f32 = mybir.dt.float32
    f16 = mybir.dt.float16
    nc = bacc.Bacc("TRN2", target_bir_lowering=False, debug=False,
                   num_devices=NCORES)
    # two logical tiles per DMA: 826 KB transfers with 6464 B lines keep
    # both HWDGE queue rings saturated from the first issue
    a = nc.declare_dram_parameter("a", [T // 2, 128, 2 * TW], f16,
                                  isOutput=False)
    y = nc.declare_dram_parameter("y", [8, 800], f32, isOutput=True)

    with TileContext(nc) as tc:
        with (
            tc.tile_pool(name="apool", bufs=T // 2) as apool,
            tc.tile_pool(name="opool", bufs=1) as opool,
            tc.tile_pool(name="ps", bufs=2, space="PSUM") as pspool,
        ):
            # full-partition tiles so each lands at PSUM base partition 0
            # (matmul output base partition must be 0/32/64/96)
            ps_a = pspool.tile([128, 400], f32, name="ps_a", tag="ps_a")[0:8, :]
            ps_b = pspool.tile([128, 400], f32, name="ps_b", tag="ps_b")[0:8, :]

            # PE_HAM releases the clock gate (1.2 -> 2.4 GHz) only after
            # ~3.4 us of sustained PE activity.  Spin dummy matmuls on a
            # zeroed tile while the first DMAs are in flight so the real
            # matmuls run warm.
            warm = apool.tile([128, 512], f16, name="warm", tag="warm")
            nc.gpsimd.memset(warm, 0.0)
            ps_w = pspool.tile([128, 512], f32, name="ps_w", tag="ps_w")[0:8, :]
            for _ in range(10):
                nc.tensor.matmul(ps_w, warm[:, 0:8], warm[:, 0:512],
                                 start=True, stop=True)
            for t in range(T // 2):
                at = apool.tile([128, 2 * TW], f16)
                eng = nc.sync if t % 2 == 0 else nc.scalar
                eng.dma_start(out=at, in_=a[t])

                for s in range(2):
                    base = s * TW

                    def mA(ps, qoff, rhs0, start=False, stop=False):
                        nc.tensor.matmul(
                            ps, at[:, base + qoff:base + qoff + 8],
                            at[:, base + rhs0:base + rhs0 + 400],
                            start=start, stop=stop)

                    def mB(ps, qoff, rhs0):
                        nc.tensor.matmul(
                            ps[0:4, :], at[:, base + qoff:base + qoff + 4],
                            at[:, base + rhs0:base + rhs0 + 400],
                            start=False, stop=False)

                    # ps_a <- even chunk pair of the sub-tile; ps_b <- odd.
                    # Per PSUM bank the first matmul carries start=True and
                    # the last carries stop=True (and must be emitted last).
                    first = t == 0 and s == 0
                    lastt = t == T // 2 - 1 and s == 1
                    if lastt:
                        mB(ps_a, 1600, 800)
                        mA(ps_a, 1600, 0, stop=True)
                        mB(ps_b, 1608, 1200)
                        mA(ps_b, 1608, 400, stop=True)
                    else:
                        mA(ps_a, 1600, 0, start=first)
                        mB(ps_a, 1600, 800)
                        mA(ps_b, 1608, 400, start=first)
                        mB(ps_b, 1608, 1200)

            yt = opool.tile([8, 800], f32)
            nc.vector.tensor_copy(out=yt[:, 0:400], in_=ps_a)
            nc.vector.tensor_copy(out=yt[:, 400:800], in_=ps_b)
            nc.sync.dma_start(out=y[:], in_=yt)
    nc.compile()
    return nc


def _split16(x):
    hi = x.astype(np.float16)
    lo = (x - hi.astype(np.float32)).astype(np.float16)
    return hi, lo


def _prep_inputs(p0, p1, I1, I2, inds1, inds2):
    idx1 = inds1.astype(np.int64).reshape(-1, 2)
    idx2 = inds2.astype(np.int64).reshape(-1, 2)
    rows1 = idx1[:, 0] * SIDE + idx1[:, 1]      # flat positions, in order
    rows2 = idx2[:, 0] * SIDE + idx2[:, 1]

    selT = np.empty((K, 2 * R), np.float32)
    selT[:, 0:R] = I1[rows1, :].T
    selT[:, R:2 * R] = I2[rows2, :].T
    sel_hi, sel_lo = _split16(selT)             # [K, 200] each
    q = np.stack([p0, p1], axis=-1)             # [K, 2]
    q_hi, q_lo = _split16(q)

    in_maps = []
    for cix in range(NCORES):
        o = cix * KS
        bh = sel_hi[o:o + KS].reshape(T, 4, 128, 2 * R)
        bl = sel_lo[o:o + KS].reshape(T, 4, 128, 2 * R)
        qh = q_hi[o:o + KS].reshape(T, 2, 2, 128, 2)
        ql = q_lo[o:o + KS].reshape(T, 2, 2, 128, 2)
        a = np.empty((T, 128, TW), np.float16)
        a[:, :, 0:200] = bh[:, 0]
        a[:, :, 200:400] = bh[:, 1]
        a[:, :, 400:600] = bh[:, 2]
        a[:, :, 600:800] = bh[:, 3]
        a[:, :, 800:1000] = bl[:, 0]
        a[:, :, 1000:1200] = bl[:, 1]
        a[:, :, 1200:1400] = bl[:, 2]
        a[:, :, 1400:1600] = bl[:, 3]
        # q groups: [1600:1604] = qhi pair0, [1604:1608] = qlo pair0,
        #           [1608:1612] = qhi pair1, [1612:1616] = qlo pair1
        a[:, :, 1600:1602] = qh[:, 0, 0]
        a[:, :, 1602:1604] = qh[:, 0, 1]
        a[:, :, 1604:1606] = ql[:, 0, 0]
        a[:, :, 1606:1608] = ql[:, 0, 1]
        a[:, :, 1608:1610] = qh[:, 1, 0]
        a[:, :, 1610:1612] = qh[:, 1, 1]
        a[:, :, 1612:1614] = ql[:, 1, 0]
        a[:, :, 1614:1616] = ql[:, 1, 1]
        # pack two logical tiles per DMA tile: [T, 128, TW] ->
        # [T//2, 128, 2*TW] with tile 2u at cols 0:TW, tile 2u+1 at TW:2*TW
        a2 = np.ascontiguousarray(
            a.reshape(T // 2, 2, 128, TW).transpose(0, 2, 1, 3)
        ).reshape(T // 2, 128, 2 * TW)
        in_maps.append({"a": a2})
    return in_maps


def _run(in_maps, trace=False):
    from concourse.bass_utils import run_bass_kernel_spmd

    if "nc" not in _cache:
        _cache["nc"] = _build_nc()
    return run_bass_kernel_spmd(
        _cache["nc"], in_maps, list(range(NCORES)), trace=trace
    )


def kernel(p0, p1, I1, I2, inds1, inds2):
    p0 = np.ascontiguousarray(np.asarray(p0, dtype=np.float32))
    p1 = np.ascontiguousarray(np.asarray(p1, dtype=np.float32))
    I1 = np.asarray(I1, dtype=np.float32)
    I2 = np.asarray(I2, dtype=np.float32)
    inds1 = np.asarray(inds1)
    inds2 = np.asarray(inds2)

    in_maps = _prep_inputs(p0, p1, I1, I2, inds1, inds2)
    results = _run(in_maps).results

    acc = np.zeros((2, R), np.float64)
    for r in results:
        yf = r["y"].astype(np.float64)
        for half in range(2):
            yc = yf[:, half * 400:(half + 1) * 400]
            acc[0] += (yc[0, 0:100] + yc[4, 0:100]
                       + yc[2, 200:300] + yc[6, 200:300])      # dgm1
            acc[1] += (yc[1, 100:200] + yc[5, 100:200]
                       + yc[3, 300:400] + yc[7, 300:400])      # dgm2
    vals = acc.astype(np.float32)
    dgm1 = vals[0].reshape(R // 2, 2)
    dgm2 = vals[1].reshape(R // 2, 2)
    return (dgm1, dgm2)
